# revision 5
# baseline (speedup 1.0000x reference)
"""Trainium2 Bass kernel for the fuzzy-rule Controller model.

Model (hardcoded; see harness reference):
  B = 1_000_000, H = 64, 8 membership nets (2 actions x 4 state features).
  x = s[:, [0,1,2,3,0,1,2,3]]
  h1 = relu(x[:,n,None] * w1[n] + b1[n])          [B, n, 64]
  h2 = relu(h1 @ w2[n] + b2[n])                   [B, n, 64]
  z  = h2 @ w3[n] + b3[n]                         [B, n]
  m  = sigmoid(z); strength = min(m, groups of 4) [B, 2]
  out = softmax(strength * 5)    (= sigmoid(+-5*(sig(minz0)-sig(minz1))))

Mapping to 8 NeuronCores: pure data parallel over batch. Each core gets
125_000 rows padded to 125_440 = 245 tiles x 512.

Per tile (T=512 batch columns), nets are processed in pairs (i, i+4),
which share state feature i, block-stacked on the PE's 128 partitions.
All matmul operands are bf16 (1 cycle/row on PE + fast weight load;
max abs output err vs fp32 reference ~6e-3, verified in numpy):
  L1: matmul lhsT=[4,128] one-hot-row w1 block, rhs=sT[4,T]   -> PSUM[128,T]
  relu(+b1)  PSUM->SBUF bf16 (ScalarE for 2 pairs, VectorE for 2)
  L2: matmul lhsT=[128,128] block-diag w2,      rhs=h1[128,T] -> PSUM[128,T]
  relu(+b2)
  L3: matmul lhsT=[128,16] (cols 2i,2i+1 = w3), rhs=h2        -> PSUM[16,T]
      all four pairs accumulate into one PSUM[16,T] (start=i==0, stop=i==3)
      (rows 8..15 are padding so the DMA-xbar transpose gets 16 rows)
  z+b3 -> SBUF bf16 (ScalarE Identity w/ bias)
  4x DMA-xbar transpose [16,128] -> [128,16]  (idle DMA engines, not PE)
  min-fold on [128, 4j x 8] batch-major, SS=sigmoid(minz), d=SS0-SS1,
  p0=sigmoid(5d) (ScalarE), p1=1-p0 (VectorE)
  staged [128, 8/tile] f32 and flushed to DRAM every 35 tiles.

Host side: transpose s shard -> sT[4, Bc] bf16; un-permute outP[128, 245*8].
"""

import sys

sys.path.insert(0, "/opt/trn_rl_repo")

from contextlib import ExitStack

import ml_dtypes
import numpy as np

import concourse.bacc as bacc
import concourse.bass as bass
import concourse.mybir as mybir
import concourse.tile as tile

F32 = mybir.dt.float32
BF16 = mybir.dt.bfloat16
AF = mybir.ActivationFunctionType
ALU = mybir.AluOpType

H = 64
N_CORES = 8
B_TOTAL = 1_000_000
B_SHARD = B_TOTAL // N_CORES  # 125_000
T = 512  # batch columns per tile (= one fp32 PSUM bank)


def _build_program(n_tiles: int, flush_tiles: int):
    """Build + compile the single-core program (SPMD: same NEFF on all cores)."""
    assert n_tiles % flush_tiles == 0
    n_groups = n_tiles // flush_tiles
    bc = n_tiles * T

    nc = bacc.Bacc("TRN2", debug=False, target_bir_lowering=False)

    sT_d = nc.dram_tensor("sT", [4, bc], BF16, kind="ExternalInput")
    w1s_d = nc.dram_tensor("w1s", [4, 512], BF16, kind="ExternalInput")
    w2s_d = nc.dram_tensor("w2s", [128, 512], BF16, kind="ExternalInput")
    w3s_d = nc.dram_tensor("w3s", [128, 64], BF16, kind="ExternalInput")
    b1s_d = nc.dram_tensor("b1s", [128, 4], F32, kind="ExternalInput")
    b2s_d = nc.dram_tensor("b2s", [128, 4], F32, kind="ExternalInput")
    b3p_d = nc.dram_tensor("b3p", [16, 1], F32, kind="ExternalInput")
    outP_d = nc.dram_tensor("outP", [128, n_tiles * 8], F32, kind="ExternalOutput")

    with tile.TileContext(nc) as tc, ExitStack() as ctx:
        wp = ctx.enter_context(tc.tile_pool(name="w", bufs=1))
        inp = ctx.enter_context(tc.tile_pool(name="in", bufs=3))
        hp = ctx.enter_context(tc.tile_pool(name="h", bufs=3))
        zp = ctx.enter_context(tc.tile_pool(name="zs", bufs=2))
        tp_ = ctx.enter_context(tc.tile_pool(name="tail", bufs=2))
        sp = ctx.enter_context(tc.tile_pool(name="stg", bufs=2))
        pA = ctx.enter_context(tc.tile_pool(name="pA", bufs=3, space="PSUM"))
        pB = ctx.enter_context(tc.tile_pool(name="pB", bufs=3, space="PSUM"))
        pZ = ctx.enter_context(tc.tile_pool(name="pZ", bufs=2, space="PSUM"))

        w1t = wp.tile([4, 512], BF16)
        nc.sync.dma_start(w1t[:], w1s_d.ap()[:])
        w2t = wp.tile([128, 512], BF16)
        nc.sync.dma_start(w2t[:], w2s_d.ap()[:])
        w3t = wp.tile([128, 64], BF16)
        nc.sync.dma_start(w3t[:], w3s_d.ap()[:])
        b1t = wp.tile([128, 4], F32)
        nc.sync.dma_start(b1t[:], b1s_d.ap()[:])
        b2t = wp.tile([128, 4], F32)
        nc.sync.dma_start(b2t[:], b2s_d.ap()[:])
        b3t = wp.tile([16, 1], F32)
        nc.sync.dma_start(b3t[:], b3p_d.ap()[:])

        for grp in range(n_groups):
            stg = sp.tile([128, flush_tiles * 8], F32)
            stgv = stg[:].rearrange("p (t j a) -> p t j a", t=flush_tiles, j=4, a=2)
            for tl in range(flush_tiles):
                t = grp * flush_tiles + tl
                st = inp.tile([4, T], BF16, tag="st")
                nc.sync.dma_start(st[:], sT_d.ap()[:, t * T : (t + 1) * T])

                zps = pZ.tile([16, T], F32)
                for i in range(4):
                    a = pA.tile([128, T], F32)
                    nc.tensor.matmul(
                        a[:],
                        w1t[:, 128 * i : 128 * (i + 1)],
                        st[:],
                        start=True,
                        stop=True,
                    )
                    h1 = hp.tile([128, T], BF16, tag="h1")
                    if i % 2 == 0:
                        nc.scalar.activation(
                            h1[:], a[:], AF.Relu, bias=b1t[:, i : i + 1]
                        )
                    else:
                        nc.vector.tensor_scalar(
                            h1[:], a[:], b1t[:, i : i + 1], 0.0, ALU.add, ALU.max
                        )
                    b = pB.tile([128, T], F32)
                    nc.tensor.matmul(
                        b[:],
                        w2t[:, 128 * i : 128 * (i + 1)],
                        h1[:],
                        start=True,
                        stop=True,
                    )
                    h2 = hp.tile([128, T], BF16, tag="h2")
                    if i % 2 == 0:
                        nc.scalar.activation(
                            h2[:], b[:], AF.Relu, bias=b2t[:, i : i + 1]
                        )
                    else:
                        nc.vector.tensor_scalar(
                            h2[:], b[:], b2t[:, i : i + 1], 0.0, ALU.add, ALU.max
                        )
                    nc.tensor.matmul(
                        zps[:],
                        w3t[:, 16 * i : 16 * (i + 1)],
                        h2[:],
                        start=(i == 0),
                        stop=(i == 3),
                    )

                # z + b3 -> SBUF bf16, then DMA-xbar transpose to batch-major
                zs = zp.tile([16, T], BF16)
                nc.scalar.activation(zs[:], zps[:], AF.Identity, bias=b3t[:])
                V = tp_.tile([128, 64], BF16, tag="V")
                for j in range(4):
                    nc.sync.dma_start_transpose(
                        V[:, 16 * j : 16 * (j + 1)],
                        zs[:, 128 * j : 128 * (j + 1)],
                    )
                # V[p, j, i, a]: batch j*128+p, pair i (i<4; 4..7 padding), action a
                V4 = V[:].rearrange("p (j i a) -> p j i a", j=4, i=8, a=2)
                M1 = tp_.tile([128, 16], BF16, tag="M1")
                M14 = M1[:].rearrange("p (j i a) -> p j i a", j=4, i=2, a=2)
                nc.vector.tensor_tensor(
                    M14, V4[:, :, 0:2, :], V4[:, :, 2:4, :], ALU.min
                )
                S = tp_.tile([128, 8], BF16, tag="S")
                S4 = S[:].rearrange("p (j a) -> p j a", j=4, a=2)
                nc.vector.tensor_tensor(
                    S4, M14[:, :, 0:1, :], M14[:, :, 1:2, :], ALU.min
                )
                SS = tp_.tile([128, 8], F32, tag="SS")
                SS4 = SS[:].rearrange("p (j a) -> p j a", j=4, a=2)
                nc.scalar.activation(SS[:], S[:], AF.Sigmoid)
                D = tp_.tile([128, 4], F32, tag="D")
                nc.vector.tensor_tensor(
                    D[:], SS4[:, :, 0:1], SS4[:, :, 1:2], ALU.subtract
                )
                # softmax over 2 actions: p0 = sigmoid(5d), p1 = 1 - p0
                nc.scalar.activation(
                    stgv[:, tl, :, 0:1], D[:], AF.Sigmoid, scale=5.0
                )
                nc.vector.tensor_scalar(
                    stgv[:, tl, :, 1:2],
                    stgv[:, tl, :, 0:1],
                    -1.0,
                    1.0,
                    ALU.mult,
                    ALU.add,
                )
            nc.sync.dma_start(
                outP_d.ap()[:, grp * flush_tiles * 8 : (grp + 1) * flush_tiles * 8],
                stg[:],
            )

    nc.compile()
    return nc


def _pack_weights(w1, b1, w2, b2, w3, b3):
    w1 = np.asarray(w1, np.float32)
    b1 = np.asarray(b1, np.float32)
    w2 = np.asarray(w2, np.float32)
    b2 = np.asarray(b2, np.float32)
    w3 = np.asarray(w3, np.float32)
    b3 = np.asarray(b3, np.float32)
    w1s = np.zeros((4, 512), np.float32)
    w2s = np.zeros((128, 512), np.float32)
    w3s = np.zeros((128, 64), np.float32)
    b1s = np.zeros((128, 4), np.float32)
    b2s = np.zeros((128, 4), np.float32)
    b3p = np.zeros((16, 1), np.float32)
    for i in range(4):
        w1s[i, 128 * i : 128 * i + 64] = w1[i]
        w1s[i, 128 * i + 64 : 128 * (i + 1)] = w1[i + 4]
        w2s[0:64, 128 * i : 128 * i + 64] = w2[i]
        w2s[64:128, 128 * i + 64 : 128 * (i + 1)] = w2[i + 4]
        w3s[0:64, 16 * i + 2 * i] = w3[i]
        w3s[64:128, 16 * i + 2 * i + 1] = w3[i + 4]
        b1s[0:64, i] = b1[i]
        b1s[64:128, i] = b1[i + 4]
        b2s[0:64, i] = b2[i]
        b2s[64:128, i] = b2[i + 4]
        b3p[2 * i, 0] = b3[i]
        b3p[2 * i + 1, 0] = b3[i + 4]
    bf = ml_dtypes.bfloat16
    return dict(
        w1s=w1s.astype(bf),
        w2s=w2s.astype(bf),
        w3s=w3s.astype(bf),
        b1s=b1s,
        b2s=b2s,
        b3p=b3p,
    )


def _make_in_maps(s, weights, n_tiles):
    s = np.asarray(s, np.float32)
    bc = n_tiles * T
    in_maps = []
    for c in range(N_CORES):
        shard = s[c * B_SHARD : (c + 1) * B_SHARD]
        sT = np.zeros((4, bc), ml_dtypes.bfloat16)
        sT[:, : shard.shape[0]] = shard.T.astype(ml_dtypes.bfloat16)
        in_maps.append(dict(weights, sT=np.ascontiguousarray(sT)))
    return in_maps


def _unpack_out(results, n_tiles):
    bc = n_tiles * T
    out = np.empty((B_TOTAL, 2), np.float32)
    for c in range(N_CORES):
        outP = results[c]["outP"]  # [128, n_tiles*8]
        full = (
            outP.reshape(128, n_tiles, 4, 2)
            .transpose(1, 2, 0, 3)
            .reshape(bc, 2)
        )
        out[c * B_SHARD : (c + 1) * B_SHARD] = full[:B_SHARD]
    return out


_NC_CACHE = {}


def _get_program(n_tiles=245, flush_tiles=35):
    key = (n_tiles, flush_tiles)
    if key not in _NC_CACHE:
        _NC_CACHE[key] = _build_program(n_tiles, flush_tiles)
    return _NC_CACHE[key]


def run(s, w1, b1, w2, b2, w3, b3, trace=False, n_tiles=245, flush_tiles=35):
    from concourse.bass_utils import run_bass_kernel_spmd

    nc = _get_program(n_tiles, flush_tiles)
    weights = _pack_weights(w1, b1, w2, b2, w3, b3)
    in_maps = _make_in_maps(s, weights, n_tiles)
    res = run_bass_kernel_spmd(
        nc, in_maps, core_ids=list(range(N_CORES)), trace=trace
    )
    return _unpack_out(res.results, n_tiles), res


def kernel(s, w1, b1, w2, b2, w3, b3):
    out, _ = run(s, w1, b1, w2, b2, w3, b3)
    return out


# revision 9
# speedup vs baseline: 2.1004x; 2.1004x over previous
"""Trainium2 Bass kernel for the fuzzy-rule Controller model.

Model (hardcoded; see harness reference):
  B = 1_000_000, H = 64, 8 membership nets (2 actions x 4 state features).
  x = s[:, [0,1,2,3,0,1,2,3]]
  h1 = relu(x[:,n,None] * w1[n] + b1[n])          [B, n, 64]
  h2 = relu(h1 @ w2[n] + b2[n])                   [B, n, 64]
  z  = h2 @ w3[n] + b3[n]                         [B, n]
  m  = sigmoid(z); strength = min(m, groups of 4) [B, 2]
  out = softmax(strength * 5)    (= sigmoid(+-5*(sig(minz0)-sig(minz1))))

Mapping to 8 NeuronCores: pure data parallel over batch. Each core gets
125_000 rows padded to 125_440 = 245 tiles x 512.

Per tile (T=512 batch columns), nets are processed in pairs (i, i+4),
which share state feature i, block-stacked on the PE's 128 partitions.
All matmul operands are bf16 (1 cycle/row on PE + fast weight load;
max abs output err vs fp32 reference ~6e-3, verified in numpy):
  L1: matmul lhsT=[4,128] one-hot-row w1 block, rhs=sT[4,T]   -> PSUM[128,T]
  relu(+b1)  PSUM->SBUF bf16 (ScalarE for 2 pairs, VectorE for 2)
  L2: matmul lhsT=[128,128] block-diag w2,      rhs=h1[128,T] -> PSUM[128,T]
  relu(+b2)
  L3: matmul lhsT=[128,16] (cols 2i,2i+1 = w3), rhs=h2        -> PSUM[16,T]
      all four pairs accumulate into one PSUM[16,T] (start=i==0, stop=i==3)
      (rows 8..15 are padding so the DMA-xbar transpose gets 16 rows)
  z+b3 -> SBUF bf16 (ScalarE Identity w/ bias)
  4x DMA-xbar transpose [16,128] -> [128,16]  (idle DMA engines, not PE)
  min-fold on [128, 4j x 8] batch-major, SS=sigmoid(minz), d=SS0-SS1,
  p0=sigmoid(5d) (ScalarE), p1=1-p0 (VectorE)
  staged [128, 8/tile] f32 and flushed to DRAM every 35 tiles.

Host side: transpose s shard -> sT[4, Bc] bf16; un-permute outP[128, 245*8].
"""

import sys

sys.path.insert(0, "/opt/trn_rl_repo")

from contextlib import ExitStack

import ml_dtypes
import numpy as np

import concourse.bacc as bacc
import concourse.bass as bass
import concourse.mybir as mybir
import concourse.tile as tile

F32 = mybir.dt.float32
BF16 = mybir.dt.bfloat16
AF = mybir.ActivationFunctionType
ALU = mybir.AluOpType

H = 64
N_CORES = 8
B_TOTAL = 1_000_000
B_SHARD = B_TOTAL // N_CORES  # 125_000
T = 512  # batch columns per tile (= one fp32 PSUM bank)


def _build_program(n_tiles: int, flush_tiles: int):
    """Build + compile the single-core program (SPMD: same NEFF on all cores)."""
    assert n_tiles % flush_tiles == 0
    n_groups = n_tiles // flush_tiles
    bc = n_tiles * T

    nc = bacc.Bacc("TRN2", debug=False, target_bir_lowering=False)

    sT_d = nc.dram_tensor("sT", [4, bc], BF16, kind="ExternalInput")
    w1s_d = nc.dram_tensor("w1s", [4, 512], BF16, kind="ExternalInput")
    w2s_d = nc.dram_tensor("w2s", [128, 512], BF16, kind="ExternalInput")
    w3s_d = nc.dram_tensor("w3s", [128, 64], BF16, kind="ExternalInput")
    b1s_d = nc.dram_tensor("b1s", [128, 4], F32, kind="ExternalInput")
    b2s_d = nc.dram_tensor("b2s", [128, 4], F32, kind="ExternalInput")
    b3p_d = nc.dram_tensor("b3p", [16, 1], F32, kind="ExternalInput")
    id16_d = nc.dram_tensor("id16", [16, 16], BF16, kind="ExternalInput")
    outP_d = nc.dram_tensor("outP", [128, n_tiles * 8], F32, kind="ExternalOutput")

    with tile.TileContext(nc) as tc, ExitStack() as ctx:
        wp = ctx.enter_context(tc.tile_pool(name="w", bufs=1))
        inp = ctx.enter_context(tc.tile_pool(name="in", bufs=3))
        hp = ctx.enter_context(tc.tile_pool(name="h", bufs=3))
        zp = ctx.enter_context(tc.tile_pool(name="zs", bufs=2))
        tp_ = ctx.enter_context(tc.tile_pool(name="tail", bufs=2))
        sp = ctx.enter_context(tc.tile_pool(name="stg", bufs=2))
        pA = ctx.enter_context(tc.tile_pool(name="pA", bufs=4, space="PSUM"))
        pB = ctx.enter_context(tc.tile_pool(name="pB", bufs=2, space="PSUM"))
        pZ = ctx.enter_context(tc.tile_pool(name="pZ", bufs=1, space="PSUM"))
        pT = ctx.enter_context(tc.tile_pool(name="pT", bufs=1, space="PSUM"))

        w1t = wp.tile([4, 512], BF16)
        nc.sync.dma_start(w1t[:], w1s_d.ap()[:])
        w2t = wp.tile([128, 512], BF16)
        nc.sync.dma_start(w2t[:], w2s_d.ap()[:])
        w3t = wp.tile([128, 64], BF16)
        nc.sync.dma_start(w3t[:], w3s_d.ap()[:])
        b1t = wp.tile([128, 4], F32)
        nc.sync.dma_start(b1t[:], b1s_d.ap()[:])
        b2t = wp.tile([128, 4], F32)
        nc.sync.dma_start(b2t[:], b2s_d.ap()[:])
        b3t = wp.tile([16, 1], F32)
        nc.sync.dma_start(b3t[:], b3p_d.ap()[:])

        id16 = wp.tile([16, 16], BF16)
        nc.sync.dma_start(id16[:], id16_d.ap()[:])

        prev = None  # (zs, stgv, tl) of the previous tile, tail not yet emitted

        def emit_tail(pz, stgv_, tl_):
            V = tp_.tile([128, 64], BF16, tag="V")
            tpm = pT.tile([128, 64], BF16)
            for j in range(4):
                nc.tensor.transpose(
                    tpm[:, 16 * j : 16 * (j + 1)],
                    pz[:, 128 * j : 128 * (j + 1)],
                    id16[:],
                )
            nc.vector.tensor_copy(V[:], tpm[:])
            # V[p, j, i, a]: batch j*128+p, pair i (i<4; 4..7 pad), action a
            V4 = V[:].rearrange("p (j i a) -> p j i a", j=4, i=8, a=2)
            M1 = tp_.tile([128, 16], BF16, tag="M1")
            M14 = M1[:].rearrange("p (j i a) -> p j i a", j=4, i=2, a=2)
            nc.vector.tensor_tensor(M14, V4[:, :, 0:2, :], V4[:, :, 2:4, :], ALU.min)
            S = tp_.tile([128, 8], BF16, tag="S")
            S4 = S[:].rearrange("p (j a) -> p j a", j=4, a=2)
            nc.vector.tensor_tensor(S4, M14[:, :, 0:1, :], M14[:, :, 1:2, :], ALU.min)
            SS = tp_.tile([128, 8], F32, tag="SS")
            SS4 = SS[:].rearrange("p (j a) -> p j a", j=4, a=2)
            nc.scalar.activation(SS[:], S[:], AF.Sigmoid)
            D = tp_.tile([128, 4], F32, tag="D")
            nc.vector.tensor_tensor(D[:], SS4[:, :, 0:1], SS4[:, :, 1:2], ALU.subtract)
            nc.scalar.activation(stgv_[:, tl_, :, 0:1], D[:], AF.Sigmoid, scale=5.0)
            nc.vector.tensor_scalar(
                stgv_[:, tl_, :, 1:2], stgv_[:, tl_, :, 0:1], -1.0, 1.0,
                ALU.mult, ALU.add,
            )

        stg = None
        for grp in range(n_groups):
            prev_stg = stg
            stg = sp.tile([128, flush_tiles * 8], F32)
            stgv = stg[:].rearrange("p (t j a) -> p t j a", t=flush_tiles, j=4, a=2)
            for tl in range(flush_tiles):
                t = grp * flush_tiles + tl
                st = inp.tile([4, T], BF16, tag="st")
                nc.sync.dma_start(st[:], sT_d.ap()[:, t * T : (t + 1) * T])

                # phase 1: all four L1 matmuls
                pa = [pA.tile([128, T], F32, tag="pa", name=f"pa{t}_{k}") for k in range(4)]
                for i in range(4):
                    nc.tensor.matmul(
                        pa[i][:], w1t[:, 128 * i : 128 * (i + 1)], st[:],
                        start=True, stop=True,
                    )
                # previous tile's transposes + tail run while relus catch up
                if prev is not None:
                    emit_tail(*prev)
                    prev = None
                if tl == 0 and prev_stg is not None:
                    nc.sync.dma_start(
                        outP_d.ap()[
                            :, (grp - 1) * flush_tiles * 8 : grp * flush_tiles * 8
                        ],
                        prev_stg[:],
                    )
                    prev_stg = None
                # relus for layer 1
                h1 = []
                for i in range(4):
                    h = hp.tile([128, T], BF16, tag="h1")
                    if i % 2 == 0:
                        nc.scalar.activation(h[:], pa[i][:], AF.Relu, bias=b1t[:, i : i + 1])
                    else:
                        nc.vector.tensor_scalar(
                            h[:], pa[i][:], b1t[:, i : i + 1], 0.0, ALU.add, ALU.max
                        )
                    h1.append(h)
                # phase 2: all four L2 matmuls + relus
                h2 = []
                pb = [pB.tile([128, T], F32, tag="pb", name=f"pb{t}_{k}") for k in range(4)]
                for i in range(4):
                    nc.tensor.matmul(
                        pb[i][:], w2t[:, 128 * i : 128 * (i + 1)], h1[i][:],
                        start=True, stop=True,
                    )
                for i in range(4):
                    h = hp.tile([128, T], BF16, tag="h2")
                    if i % 2 == 0:
                        nc.scalar.activation(h[:], pb[i][:], AF.Relu, bias=b2t[:, i : i + 1])
                    else:
                        nc.vector.tensor_scalar(
                            h[:], pb[i][:], b2t[:, i : i + 1], 0.0, ALU.add, ALU.max
                        )
                    h2.append(h)
                # phase 3: L3 accumulating matmuls -> z, then z+b3 -> SBUF bf16
                zps = pZ.tile([16, T], F32)
                for i in range(4):
                    nc.tensor.matmul(
                        zps[:], w3t[:, 16 * i : 16 * (i + 1)], h2[i][:],
                        start=(i == 0), stop=(i == 3),
                    )
                zs = zp.tile([16, T], BF16)
                nc.scalar.activation(zs[:], zps[:], AF.Identity, bias=b3t[:])
                prev = (zs, stgv, tl)
        # drain the last tile's tail + final flush
        emit_tail(*prev)
        nc.sync.dma_start(
            outP_d.ap()[:, (n_groups - 1) * flush_tiles * 8 : n_groups * flush_tiles * 8],
            stg[:],
        )

    nc.compile()
    return nc


def _pack_weights(w1, b1, w2, b2, w3, b3):
    w1 = np.asarray(w1, np.float32)
    b1 = np.asarray(b1, np.float32)
    w2 = np.asarray(w2, np.float32)
    b2 = np.asarray(b2, np.float32)
    w3 = np.asarray(w3, np.float32)
    b3 = np.asarray(b3, np.float32)
    w1s = np.zeros((4, 512), np.float32)
    w2s = np.zeros((128, 512), np.float32)
    w3s = np.zeros((128, 64), np.float32)
    b1s = np.zeros((128, 4), np.float32)
    b2s = np.zeros((128, 4), np.float32)
    b3p = np.zeros((16, 1), np.float32)
    for i in range(4):
        w1s[i, 128 * i : 128 * i + 64] = w1[i]
        w1s[i, 128 * i + 64 : 128 * (i + 1)] = w1[i + 4]
        w2s[0:64, 128 * i : 128 * i + 64] = w2[i]
        w2s[64:128, 128 * i + 64 : 128 * (i + 1)] = w2[i + 4]
        w3s[0:64, 16 * i + 2 * i] = w3[i]
        w3s[64:128, 16 * i + 2 * i + 1] = w3[i + 4]
        b1s[0:64, i] = b1[i]
        b1s[64:128, i] = b1[i + 4]
        b2s[0:64, i] = b2[i]
        b2s[64:128, i] = b2[i + 4]
        b3p[2 * i, 0] = b3[i]
        b3p[2 * i + 1, 0] = b3[i + 4]
    bf = ml_dtypes.bfloat16
    return dict(
        w1s=w1s.astype(bf),
        w2s=w2s.astype(bf),
        w3s=w3s.astype(bf),
        b1s=b1s,
        b2s=b2s,
        b3p=b3p,
        id16=np.eye(16, dtype=ml_dtypes.bfloat16),
    )


def _make_in_maps(s, weights, n_tiles):
    s = np.asarray(s, np.float32)
    bc = n_tiles * T
    in_maps = []
    for c in range(N_CORES):
        shard = s[c * B_SHARD : (c + 1) * B_SHARD]
        sT = np.zeros((4, bc), ml_dtypes.bfloat16)
        sT[:, : shard.shape[0]] = shard.T.astype(ml_dtypes.bfloat16)
        in_maps.append(dict(weights, sT=np.ascontiguousarray(sT)))
    return in_maps


def _unpack_out(results, n_tiles):
    bc = n_tiles * T
    out = np.empty((B_TOTAL, 2), np.float32)
    for c in range(N_CORES):
        outP = results[c]["outP"]  # [128, n_tiles*8]
        full = (
            outP.reshape(128, n_tiles, 4, 2)
            .transpose(1, 2, 0, 3)
            .reshape(bc, 2)
        )
        out[c * B_SHARD : (c + 1) * B_SHARD] = full[:B_SHARD]
    return out


_NC_CACHE = {}


def _get_program(n_tiles=245, flush_tiles=35):
    key = (n_tiles, flush_tiles)
    if key not in _NC_CACHE:
        _NC_CACHE[key] = _build_program(n_tiles, flush_tiles)
    return _NC_CACHE[key]


def run(s, w1, b1, w2, b2, w3, b3, trace=False, n_tiles=245, flush_tiles=35):
    from concourse.bass_utils import run_bass_kernel_spmd

    nc = _get_program(n_tiles, flush_tiles)
    weights = _pack_weights(w1, b1, w2, b2, w3, b3)
    in_maps = _make_in_maps(s, weights, n_tiles)
    res = run_bass_kernel_spmd(
        nc, in_maps, core_ids=list(range(N_CORES)), trace=trace
    )
    return _unpack_out(res.results, n_tiles), res


def kernel(s, w1, b1, w2, b2, w3, b3):
    out, _ = run(s, w1, b1, w2, b2, w3, b3)
    return out
